# revision 5
# baseline (speedup 1.0000x reference)
"""Trainium2 Bass kernel for the block-diagonal grouped linear
(e3nn-style per-l channel mixing):

    out[:, l^2:l^2+2l+1, :] = path_weights[l] * x[:, l^2:..., :] @ weights[l]

Strategy: data-parallel over the node axis (8 cores x 6250 nodes), with
all streaming I/O in fp16 (tolerance is 2e-2; fp16 contributes ~5e-4).
Host folds path_weights into the weights, casts x to fp16 and transposes
each l-block to [c_in=128, rows_l], concatenating the 4 blocks into one
[128, 100000] stream per core.  On device the kernel is a pure stream:

    DMA-in 2MB fp16 chunks -> fp16 matmul per 512 cols (W_l stationary,
    windows split at l boundaries) -> VectorE copy PSUM(f32)->SBUF(fp16)
    -> DMA-out fp16.

Loads ride the SP HWDGE ring, stores the ACT HWDGE ring, constants the
SWDGE (gpsimd) ring, so both HBM directions stream concurrently at the
~358 GB/s per-core HBM limit -> ~51.2 MB / core -> ~140 us.
"""

import os
import sys
import types

if "/opt/trn_rl_repo" not in sys.path:
    sys.path.insert(0, "/opt/trn_rl_repo")

import numpy as np

N_CORES = 8
N_NODES = 50000
LMAX = 3
CH = 128
NPC = N_NODES // N_CORES  # nodes per core
ROWS = [NPC * (2 * l + 1) for l in range(LMAX + 1)]  # cols per l per core
TOT = sum(ROWS)  # 100000
# column boundaries of the l segments within the concatenated stream
BOUND = [0]
for r in ROWS:
    BOUND.append(BOUND[-1] + r)

CHUNK = int(os.environ.get("K_CHUNK", "8192"))  # cols per load chunk (2 MiB fp16)
SUB = int(os.environ.get("K_SUB", "8192"))  # cols per store subchunk
IOBUFS = int(os.environ.get("K_IOBUFS", "5"))
OUTBUFS = int(os.environ.get("K_OUTBUFS", "4"))
WARMUP_MM = int(os.environ.get("K_WARMUP_MM", "12"))
# moving free dim per matmul (one PSUM bank fp32; ISA caps moving at 512)
MM = int(os.environ.get("K_MM", "512"))
PSBUFS = int(os.environ.get("K_PSBUFS", "8"))

# load-chunk schedule: small first chunk so the first store fires early
# (store lags load by one chunk's compute), then full chunks.
def _sched():
    first = min(2048, CHUNK)
    s = [first]
    if CHUNK - first > 0:
        s.append(CHUNK - first)
    left = TOT - sum(s)
    while left > 0:
        c = min(CHUNK, left)
        s.append(c)
        left -= c
    assert sum(s) == TOT
    return s

SCHED = _sched()

_nc = None  # compiled Bass program, cached across kernel() calls
LAST_RESULTS = None  # BassKernelResults of the last run (for test harnesses)


def _install_ntff_hook():
    """Make trace=True work under axon: register the NTFF profile hook the
    image's antenv package is missing.  Harmless if anything is absent."""
    try:
        import antenv

        if "antenv.axon_hooks" in sys.modules:
            return
        mod = types.ModuleType("antenv.axon_hooks")
        mod._hook = None

        def set_axon_ntff_profile_hook(h):
            mod._hook = h

        def get_axon_ntff_profile_hook():
            return mod._hook

        mod.set_axon_ntff_profile_hook = set_axon_ntff_profile_hook
        mod.get_axon_ntff_profile_hook = get_axon_ntff_profile_hook
        sys.modules["antenv.axon_hooks"] = mod
        antenv.axon_hooks = mod

        from trn_agent_boot.trn_boot import _ntff_profile_via_ctypes

        hook = _ntff_profile_via_ctypes("/opt/axon/libaxon_pjrt.so")
        if hook is not None:
            set_axon_ntff_profile_hook(hook)
    except Exception:
        pass


def _l_of_col(c):
    for l in range(LMAX + 1):
        if c < BOUND[l + 1]:
            return l
    raise ValueError(c)


def _build():
    import concourse.bacc as bacc
    import concourse.mybir as mybir
    import concourse.tile as tile

    f16 = mybir.dt.float16
    f32 = mybir.dt.float32

    nc = bacc.Bacc(
        "TRN2", target_bir_lowering=False, debug=False, num_devices=N_CORES
    )

    xt = nc.dram_tensor("xt", [CH, TOT], f16, kind="ExternalInput").ap()
    w = nc.dram_tensor("w", [(LMAX + 1) * CH, CH], f16, kind="ExternalInput").ap()
    outT = nc.dram_tensor("outT", [CH, TOT], f16, kind="ExternalOutput").ap()

    with tile.TileContext(nc) as tc:
        with (
            tc.tile_pool(name="const", bufs=1) as cpool,
            tc.tile_pool(name="io", bufs=IOBUFS) as iopool,
            tc.tile_pool(name="psum", bufs=1, space="PSUM") as pspool,
        ):
            # Constants preload on the ACT (store) HWDGE ring, which is idle
            # until the first store -- so w_sb is resident before the first
            # xt chunk lands on the SP ring.
            w_sb = cpool.tile([CH, LMAX + 1, CH], f16)
            for l in range(LMAX + 1):
                nc.scalar.dma_start(w_sb[:, l, :], w[l * CH : (l + 1) * CH, :])

            # PSUM banks are reused round-robin from a fixed set of tiles
            # (not a rotating pool) to keep the total tile count -- and the
            # Tile epilogue's per-tile teardown churn -- small.
            ps_tiles = [
                pspool.tile([CH, MM], f32, tag=f"ps{i}", name=f"ps{i}")
                for i in range(PSBUFS)
            ]

            # PE warm-up: the HAM clock gate keeps the PE at half rate until
            # ~4us of sustained matmul activity.  Burn that in during the
            # startup shadow (before the first xt chunk lands) with dummy
            # matmuls on a zeroed tile, so the real stream runs full rate.
            if WARMUP_MM:
                wu_sb = cpool.tile([CH, MM], f16)
                nc.vector.memset(wu_sb[:, :], 0.0)
                for i in range(WARMUP_MM):
                    # rotate over the real psum tiles; the WAW deps both
                    # serialize the burst (continuous PE activity) and
                    # cleanly order it before the first real windows.
                    nc.tensor.matmul(
                        ps_tiles[i % PSBUFS][:, :],
                        wu_sb[:, :CH],
                        wu_sb[:, :],
                        start=True,
                        stop=True,
                    )

            widx = 0  # global window counter, for engine round-robin
            sidx = 0  # store counter, for store-queue round-robin
            j0 = 0
            for cw in SCHED:
                xt_sb = iopool.tile([CH, CHUNK], f16, tag="xt")
                nc.sync.dma_start(xt_sb[:, :cw], xt[:, j0 : j0 + cw])
                # stores fire per SUB-col subchunk (own tile each, so the
                # store's dependency is just that subchunk's copies)
                for s0 in range(0, cw, SUB):
                    sw = min(SUB, cw - s0)
                    out_sb = iopool.tile([CH, SUB], f16, tag="out", bufs=OUTBUFS)
                    for k0 in range(s0, s0 + sw, MM):
                        n = min(MM, s0 + sw - k0)
                        ps = ps_tiles[widx % PSBUFS]
                        # split the window at l-segment boundaries (at most
                        # one boundary per window in practice)
                        s = j0 + k0
                        while s < j0 + k0 + n:
                            l = _l_of_col(s)
                            e = min(BOUND[l + 1], j0 + k0 + n)
                            a, b = s - j0, e - j0  # chunk-local col range
                            nc.tensor.matmul(
                                ps[:, a - k0 : b - k0],
                                w_sb[:, l, :],
                                xt_sb[:, a:b],
                                start=True,
                                stop=True,
                            )
                            s = e
                        # PSUM -> SBUF drain alternates DVE / ACT so neither
                        # engine becomes the bottleneck.
                        if widx % 2 == 0:
                            nc.vector.tensor_scalar_mul(
                                out_sb[:, k0 - s0 : k0 - s0 + n], ps[:, :n], 1.0
                            )
                        else:
                            nc.scalar.copy(
                                out_sb[:, k0 - s0 : k0 - s0 + n], ps[:, :n]
                            )
                        widx += 1
                    # Stores alternate between the ACT HWDGE ring and the
                    # SWDGE (gpsimd) ring so two store queues interleave at
                    # the SDMA level; the first two stay on ACT because
                    # SWDGE needs ~12us to come up.
                    s_eng = nc.scalar if (sidx < 2 or sidx % 2 == 0) else nc.gpsimd
                    s_eng.dma_start(
                        outT[:, j0 + s0 : j0 + s0 + sw], out_sb[:, :sw]
                    )
                    sidx += 1
                j0 += cw

    nc.compile()
    return nc


def kernel(x, weights, path_weights):
    global _nc, LAST_RESULTS
    _install_ntff_hook()
    from concourse.bass_utils import run_bass_kernel_spmd

    if _nc is None:
        _nc = _build()

    x = np.asarray(x, dtype=np.float32)
    weights = np.asarray(weights, dtype=np.float32)
    path_weights = np.asarray(path_weights, dtype=np.float32)

    # fold the per-path scale into the weights (both are tiny constants)
    w_eff = weights * path_weights[:, None, None]
    w_flat = np.ascontiguousarray(
        w_eff.reshape((LMAX + 1) * CH, CH), dtype=np.float16
    )

    x16 = x.astype(np.float16)
    in_maps = []
    for c in range(N_CORES):
        xc = x16[c * NPC : (c + 1) * NPC]  # [NPC, 16, CH] fp16
        cols = np.empty((CH, TOT), dtype=np.float16)
        for l in range(LMAX + 1):
            s, wd = l * l, 2 * l + 1
            cols[:, BOUND[l] : BOUND[l + 1]] = (
                xc[:, s : s + wd, :].reshape(NPC * wd, CH).T
            )
        in_maps.append({"xt": cols, "w": w_flat})

    res = run_bass_kernel_spmd(_nc, in_maps, core_ids=list(range(N_CORES)))
    LAST_RESULTS = res

    out = np.empty((N_NODES, (LMAX + 1) ** 2, CH), dtype=np.float32)
    for c in range(N_CORES):
        oc = res.results[c]["outT"]  # [CH, TOT] fp16
        for l in range(LMAX + 1):
            s, wd = l * l, 2 * l + 1
            out[c * NPC : (c + 1) * NPC, s : s + wd, :] = (
                oc[:, BOUND[l] : BOUND[l + 1]].T.reshape(NPC, wd, CH)
            ).astype(np.float32)
    return out
